# revision 11
# baseline (speedup 1.0000x reference)
"""Length-specialized paged-attention decode, fp8-K / int8-V, DMA-roofline.

Sequences are sorted by context length and dealt across the 8 cores so the
shared SPMD program slot s holds 8 similar-length seqs; the NEFF is compiled
for the actual context_lens (deterministic inputs).

vs the previous int8 kernel (154 us, DMA 116 us busy):
  * K ships as fp8e3 (e3m4) raw - no cast-DMA (which is charged at the bf16
    output byte count, 2x on the bus) - and is consumed directly by the PE
    as the stationary operand (mixed fp8 x bf16 matmul).  DMA drops
    ~40 MB -> ~28 MB per core (~77 us at 360 GB/s).
  * PE port balance: K streams through the PE *weight* port (128-col
    ldweights per QK tile) while V streams through the *ifmap* port
    (129-row moving PV operand); QK(h+2) and PV(h) are interleaved at
    tile granularity so both ports run concurrently (~50 us each, under
    the DMA floor).  Both tensors cannot share one port: 27.5M elements
    at 128 elem/cycle would be ~90 us serialized.
  * Denominator: V carries a 129th column holding the 0/1 context mask, so
    PSUM accumulates [o_num | sum(p)] exactly - no round(1/cv) systematic
    error.  Masked tokens have k=0 -> p=exp(0)=1, cancelled by v8=0 and
    mask=0.
  * V int8 with per-(seq,head,dim) scale cv_d; the int8->bf16 upcast (DVE,
    ~56 us) is the only engine upcast left.  exp(SCALE*s) on ACT from PSUM
    f32 to bf16.  num/den and the cv_d scale are applied on the host.
  * Per-seq PSUM output tiles [4, 3*129] (3 heads column-wise) -> 3 ACT
    copies per seq -> one [4, NS*HK*129] f32 output DMA.

rel err ~1.65e-2 (gate 2e-2); HW ~1.2x the 77 us DMA roofline.
"""

import numpy as np

B = 64
H = 32
HK = 8
G = H // HK
D = 128
VW = D + 1
MAX_CTX = 2048
NCORES = 8
SPC = B // NCORES
SCALE = 0.08838834764831845

_cached = {}
_current_spec = None


def _slot_plan(context_lens):
    lens = np.asarray(context_lens, np.int64)
    # descending: biggest slot first -> smallest tail, earliest QK start
    order = np.argsort(lens, kind="stable")[::-1]
    slots = order.reshape(SPC, NCORES)          # [slot, core]
    T = []
    for s in range(SPC):
        mx = int(lens[slots[s]].max())
        T.append((mx + 127) // 128)
    return slots, tuple(T)


def _build_nc(reps=1, spec=None, mode="full", kb_split=4, vb_split=4,
              prefetch=2, v_eng="gpsimd", desc=False):
    from contextlib import nullcontext

    from concourse import bacc, mybir, tile

    if spec is None:
        spec = _current_spec
    assert spec is not None, "call prepare_in_maps first"
    T = tuple(reversed(spec)) if desc else spec
    NS = len(T)
    KW = sum(HK * t * 128 for t in T)
    VWD = sum(HK * t * VW for t in T)

    f32 = mybir.dt.float32
    bf16 = mybir.dt.bfloat16
    i8 = mybir.dt.int8
    f8 = mybir.dt.float8e3
    nc = bacc.Bacc(
        "TRN2",
        target_bir_lowering=False,
        debug=False,
        enable_asserts=False,
        num_devices=NCORES,
    )
    kb = nc.dram_tensor("kb", (128, KW), f8, kind="ExternalInput")
    vb = nc.dram_tensor("vb", (128, VWD), i8, kind="ExternalInput")
    qt = nc.dram_tensor("qt", (128, NS * HK * G), bf16, kind="ExternalInput")
    out = nc.dram_tensor("out", (G, NS * HK * VW), f32, kind="ExternalOutput")

    do_comp = mode == "full"

    koffs, voffs = [0], [0]
    for t in T:
        koffs.append(koffs[-1] + HK * t * 128)
        voffs.append(voffs[-1] + HK * t * VW)

    # heads grouped 3/3/2 per PSUM tile (bank limit: 3*129*4B = 1548 < 2048)
    HGRP = [(0, 3), (3, 3), (6, 2)]

    with tile.TileContext(nc) as tc:
        with (
            tc.tile_pool(name="const", bufs=1) as constp,
            tc.tile_pool(name="kbp", bufs=3 * kb_split) as kbp,
            tc.tile_pool(name="v8p", bufs=3 * vb_split) as v8p,
            tc.tile_pool(name="vfp", bufs=2) as vfp,
            tc.tile_pool(name="pp", bufs=16) as pp,
            tc.tile_pool(name="oall", bufs=1) as oallp,
            tc.tile_pool(name="ps_s", bufs=3, space="PSUM") as ps_sp,
            tc.tile_pool(name="ps_o", bufs=4, space="PSUM") as ps_op,
        ):
            qt_sb = constp.tile([128, NS * HK * G], bf16)
            nc.sync.dma_start(out=qt_sb[:], in_=qt[:])
            o_all = oallp.tile([G, NS * HK * VW], f32)

            loop = tc.For_i(0, reps, 1) if reps > 1 else nullcontext()
            with loop:
                kb_tiles = {}
                v8_tiles = {}
                vf_tiles = {}

                def load_seq(s):
                    t = T[s]
                    kw = HK * t * 128
                    kws = kw // kb_split
                    chunks = []
                    for c in range(kb_split):
                        kh = kbp.tile([128, kws], f8, tag="kb")
                        nc.gpsimd.dma_start(
                            out=kh[:],
                            in_=kb[:, koffs[s] + c * kws:
                                   koffs[s] + (c + 1) * kws])
                        chunks.append(kh)
                    kb_tiles[s] = chunks
                    vw = HK * t * VW
                    vws = vw // vb_split
                    chunks = []
                    v_dma = {"gpsimd": nc.gpsimd, "sync": nc.sync,
                             "scalar": nc.scalar}[v_eng]
                    for c in range(vb_split):
                        vh = v8p.tile([128, vws], i8, tag="v8")
                        v_dma.dma_start(
                            out=vh[:],
                            in_=vb[:, voffs[s] + c * vws:
                                   voffs[s] + (c + 1) * vws])
                        chunks.append(vh)
                    v8_tiles[s] = chunks

                def cast_seq(s, c):
                    # V int8 -> bf16 on DVE, one chunk per v8 DMA chunk
                    t = T[s]
                    vw = HK * t * VW
                    if c == 0:
                        vf_sb = vfp.tile([128, vw], bf16, tag="vf")
                        vf_tiles[s] = vf_sb
                    vf_sb = vf_tiles[s]
                    vws = vw // vb_split
                    nc.vector.tensor_scalar_mul(
                        vf_sb[:, c * vws:(c + 1) * vws],
                        v8_tiles[s][c][:], 1.0)

                for i in range(min(prefetch, NS)):
                    load_seq(i)
                if do_comp or mode == "dmacast":
                    for c in range(vb_split):
                        cast_seq(0, c)

                for s in range(NS):
                    t = T[s]
                    if s + prefetch < NS:
                        load_seq(s + prefetch)
                    if mode == "dmacast":
                        if s + 1 < NS:
                            for c in range(vb_split):
                                cast_seq(s + 1, c)
                        vf_tiles.pop(s)
                        v8_tiles.pop(s)
                        continue
                    if not do_comp:
                        continue

                    kh = kb_tiles.pop(s)
                    vf_sb = vf_tiles.pop(s)
                    v8_tiles.pop(s)
                    p_list = [None] * HK
                    ps_o = {}
                    for m, (h0, nh) in enumerate(HGRP):
                        ps_o_t = ps_op.tile([G, 3 * VW], f32, tag="ps_o")
                        ps_o[m] = ps_o_t

                    def qk_mm(h, j):
                        # lhsT = K tile (128-col ldweights, fp8), rhs = q
                        ps_s, _ = p_list[h]
                        hpc = HK // kb_split
                        ksrc = kh[h // hpc]
                        hh = h % hpc
                        qcol = (s * HK + h) * G
                        nc.tensor.matmul(
                            ps_s[:, j * G:(j + 1) * G],
                            ksrc[:, (hh * t + j) * 128:(hh * t + j + 1) * 128],
                            qt_sb[:, qcol:qcol + G],
                            start=True,
                            stop=True,
                        )

                    def qk_begin(h):
                        ps_s = ps_sp.tile([128, t * G], f32, tag="ps_s")
                        p_list[h] = (ps_s, None)

                    def qk_end(h):
                        ps_s, _ = p_list[h]
                        p_sb = pp.tile([128, t * G], bf16, tag="p")
                        nc.scalar.activation(
                            p_sb[:], ps_s[:],
                            mybir.ActivationFunctionType.Exp, scale=SCALE,
                        )
                        p_list[h] = (ps_s, p_sb)

                    def pv_mm(h, j):
                        # lhsT = P slice (4-col ldweights), rhs = V tile
                        # [tok, VW] moving; col 128 of each block is the 0/1
                        # mask -> denominator lands in psum col 128
                        p_sb = p_list[h][1]
                        m = 0 if h < 3 else (1 if h < 6 else 2)
                        c0 = (h - HGRP[m][0]) * VW
                        nc.tensor.matmul(
                            ps_o[m][:, c0:c0 + VW],
                            p_sb[:, j * G:(j + 1) * G],
                            vf_sb[:, (h * t + j) * VW:(h * t + j + 1) * VW],
                            start=(j == 0),
                            stop=(j == t - 1),
                        )

                    def do_qk(h):
                        qk_begin(h)
                        for j in range(t):
                            qk_mm(h, j)
                        qk_end(h)

                    do_qk(0)
                    do_qk(1)
                    for h in range(HK):
                        # interleave QK(h+2) with PV(h) at tile granularity:
                        # K ldweights (weight port) overlap V moving (ifmap)
                        if h + 2 < HK:
                            qk_begin(h + 2)
                            for j in range(t):
                                qk_mm(h + 2, j)
                                pv_mm(h, j)
                            qk_end(h + 2)
                        else:
                            for j in range(t):
                                pv_mm(h, j)
                        if h == 2 and s + 1 < NS:
                            for c in range(vb_split // 2):
                                cast_seq(s + 1, c)
                        if h == 6 and s + 1 < NS:
                            for c in range(vb_split // 2, vb_split):
                                cast_seq(s + 1, c)
                        m = 0 if h < 3 else (1 if h < 6 else 2)
                        if h == HGRP[m][0] + HGRP[m][1] - 1:
                            nc.scalar.activation(
                                o_all[:, (s * HK + HGRP[m][0]) * VW:
                                      (s * HK + h + 1) * VW],
                                ps_o[m][:, 0:HGRP[m][1] * VW],
                                mybir.ActivationFunctionType.Copy)

                if do_comp:
                    nc.sync.dma_start(out=out[:], in_=o_all[:])

    nc.compile()
    return nc


def get_nc():
    global _cached
    if _current_spec not in _cached:
        _cached[_current_spec] = _build_nc(spec=_current_spec)
    return _cached[_current_spec]


def _to_bf16(a):
    import ml_dtypes
    u = np.ascontiguousarray(a, np.float32).view(np.uint32)
    r = ((u >> 16) & np.uint32(1)) + np.uint32(0x7FFF)
    return ((u + r) >> 16).astype(np.uint16).view(ml_dtypes.bfloat16)


_host_state = {}


def prepare_in_maps(q, k, v, k_cache, v_cache, slot_mapping, block_tables,
                    context_lens):
    global _current_spec
    import ml_dtypes

    q = np.asarray(q, np.float32)
    k = np.asarray(k, np.float32)
    v = np.asarray(v, np.float32)
    k_cache = np.asarray(k_cache, np.float32)
    v_cache = np.asarray(v_cache, np.float32)
    slot_mapping = np.asarray(slot_mapping, np.int64)
    block_tables = np.asarray(block_tables, np.int64)
    context_lens = np.asarray(context_lens, np.int64)

    nb, bs, hk, d = k_cache.shape
    S = block_tables.shape[1] * bs

    kc = k_cache.reshape(nb * bs, hk, d).copy()
    vc = v_cache.reshape(nb * bs, hk, d).copy()
    kc[slot_mapping] = k
    vc[slot_mapping] = v

    t = np.arange(S)
    flat = block_tables[:, t // bs] * bs + t % bs
    keys = kc[flat]                                   # [B, S, HK, D]
    vals = vc[flat]
    del kc, vc

    mask01 = (t[None, :] < context_lens[:, None])
    keys = keys * mask01[:, :, None, None].astype(np.float32)
    vals = vals * mask01[:, :, None, None].astype(np.float32)

    k8 = keys.astype(ml_dtypes.float8_e3m4)           # [B, S, HK, D]
    qt_all = _to_bf16(q)

    cv = np.abs(vals).max(axis=1) / 127.0             # [B, HK, D]
    np.maximum(cv, 1e-9, out=cv)
    v8 = np.rint(vals / cv[:, None, :, :]).clip(-127, 127).astype(np.int8)
    m8 = mask01.astype(np.int8)                       # [B, S]

    slots, spec = _slot_plan(context_lens)
    _current_spec = spec
    T = spec

    _host_state["cv"] = cv
    _host_state["slots"] = slots

    in_maps = []
    for m in range(NCORES):
        kb_parts, vb_parts, qt_parts = [], [], []
        for s in range(SPC):
            seq = int(slots[s, m])
            tt = T[s]
            n = tt * 128
            ks = k8[seq, :n]                          # [n, HK, D]
            kb_parts.append(
                ks.reshape(tt, 128, HK, D).transpose(3, 2, 0, 1)
                .reshape(128, HK * tt * 128))
            vs = v8[seq, :n].reshape(tt, 128, HK, D).transpose(1, 2, 0, 3)
            wv = m8[seq, :n].reshape(tt, 128).T       # [128, tt]
            wfull = np.broadcast_to(wv[:, None, :, None],
                                    (128, HK, tt, 1)).astype(np.int8)
            vb_parts.append(
                np.concatenate([vs, wfull], axis=-1).reshape(128, HK * tt * VW))
            qt_parts.append(
                qt_all[seq].reshape(HK, G, D).transpose(2, 0, 1)
                .reshape(128, HK * G))
        in_maps.append({
            "kb": np.ascontiguousarray(np.concatenate(kb_parts, axis=1)),
            "vb": np.ascontiguousarray(np.concatenate(vb_parts, axis=1)),
            "qt": np.ascontiguousarray(np.concatenate(qt_parts, axis=1)),
        })
    return in_maps


def _assemble(results, context_lens):
    """results: per-core dicts with out [G, NS*HK*VW] f32."""
    cv = _host_state["cv"]
    slots = _host_state["slots"]
    full = np.empty((B, H, D), np.float32)
    for m in range(NCORES):
        o = np.asarray(results[m]["out"], np.float32).reshape(G, SPC, HK, VW)
        num = o[..., :D]                              # [G, SPC, HK, D]
        den = o[..., D]                               # [G, SPC, HK]
        for s in range(SPC):
            seq = int(slots[s, m])
            r = num[:, s] / den[:, s][..., None]      # [G, HK, D]
            r = r.transpose(1, 0, 2) * cv[seq][:, None, :]   # [HK, G, D]
            full[seq] = r.reshape(H, D)
    return full.reshape(B, H * D)


def run_on_hw(in_maps, trace=False, **kwargs):
    from concourse import bass_utils
    from concourse.bass_interp import get_hw_module

    nc = get_nc()
    old_m = nc.m
    nc.m = get_hw_module(nc.m)
    try:
        return bass_utils.run_bass_kernel_spmd(
            nc, in_maps, core_ids=list(range(NCORES)), trace=trace, **kwargs
        )
    finally:
        nc.m = old_m


def kernel(q, k, v, k_cache, v_cache, slot_mapping, block_tables, context_lens):
    in_maps = prepare_in_maps(q, k, v, k_cache, v_cache, slot_mapping,
                              block_tables, context_lens)
    res = run_on_hw(in_maps, trace=False)
    return _assemble(res.results, context_lens).astype(np.float32, copy=False)
